# revision 41
# baseline (speedup 1.0000x reference)
"""Multi-head attention (B=4, T=2048, E=1024, H=16, D=64) on 8 TRN2 cores.

Sharding: core c handles batch b = c//2 and heads hg = c%2 (8 heads each).
No cross-device comms: each core emits a partial out-projection
y_partial[b] over its 512 head-columns; the host sums core pairs.

Per-core kernel phases:
  P: q/k/v projections (fp32r matmuls, weights stationary) + RoPE
     (rotate-half via PE permutation matmul, combine on DVE)
  A: causal attention per (q-block 512, head): scoresT[k,q] = K_h Q_h^T,
     exp on ACT (scale 1/8, additive mask on diagonal chunks),
     AV with a ones-row appended to V -> softmax denominator for free,
     divide via DVE after gpsimd partition-broadcast of the reciprocal
  O: out-projection y[t,e] with WoT stationary chunks
"""
import sys
import numpy as np
from contextlib import ExitStack

try:
    import concourse  # noqa: F401
except ImportError:
    sys.path.insert(0, "/opt/trn_rl_repo")

import concourse.tile as tile  # noqa: E402
from concourse import bacc, mybir  # noqa: E402
from concourse.bass_utils import run_bass_kernel_spmd  # noqa: E402

F32 = mybir.dt.float32
F32R = mybir.dt.float32r
AF = mybir.ActivationFunctionType

B, T, E, H, D = 4, 2048, 1024, 16, 64
N_CORES = 8
HPC = 8            # heads per core
EC = HPC * D       # 512 head-columns per core
TB = 512           # t/q block
KC = 128           # k chunk
NTB = T // TB      # 4
NTT = T // KC      # 16
CCH = E // 128     # 8 contraction chunks for x projections
OCH = EC // 128    # 4 chunks of the per-core head-column dim
ROPE_BASE = 10000.0

_NC = None


def _round_f32r(a: np.ndarray) -> np.ndarray:
    b = np.ascontiguousarray(a, dtype=np.float32).view(np.uint32)
    r = (b.astype(np.uint64) + 0x800) & 0xFFFFF000
    return r.astype(np.uint32).view(np.float32)


def _build():
    nc = bacc.Bacc("TRN2", target_bir_lowering=False, debug=False,
                   num_devices=N_CORES)
    ap = {}
    def din(name, shape, dt=F32R):
        ap[name] = nc.dram_tensor(name, shape, dt, kind="ExternalInput").ap()
    din("xT", [E, T])              # x[b].T
    din("wqT", [E, EC])            # Wq[cols,:].T
    din("wkT", [E, EC])
    din("wvT", [E, EC])
    din("woT", [EC, E])            # Wo[:,cols].T
    din("p2t", [128, 128])         # rotate-half permutation (lhsT form)
    din("cosb", [128, T], F32)     # cos dup'd over 2 heads, [2*64, T]
    din("sinb", [128, T], F32)
    y = nc.dram_tensor("y", [T, E], F32, kind="ExternalOutput").ap()

    with tile.TileContext(nc) as tc, ExitStack() as octx:
        # tensors that live across phases
        persist = octx.enter_context(tc.tile_pool(name="persist", bufs=1))
        qT = persist.tile([128, OCH, T], F32R, tag="qT")
        kT = persist.tile([128, OCH, T], F32R, tag="kT")
        vv = persist.tile([128, NTT, HPC, D + 1], F32R, tag="vv")
        wv_sb = persist.tile([128, CCH, EC], F32R, tag="wv")
        ones_sb = persist.tile([128, HPC], F32, tag="ones")
        nc.vector.memset(ones_sb[:], 1.0)

        xTr = ap["xT"].rearrange("(c p) t -> p c t", p=128)

        def emit_vproj(tt, xt_tile, xt_off, vps_pool, vtag):
            # v: out[t 128, e 512], x chunks stationary
            pp = vps_pool.tile([128, EC], F32, tag=vtag)
            for c in range(CCH):
                nc.tensor.matmul(
                    pp[:], xt_tile[:, c, xt_off:xt_off + 128],
                    wv_sb[:, c, :], start=(c == 0), stop=(c == CCH - 1))
            nc.vector.tensor_copy(
                vv[:, tt, :, 0:D], pp[:].rearrange("p (h d) -> p h d", d=D))
            nc.vector.tensor_copy(vv[:, tt, :, D], ones_sb[:])

        # ---------------- Phase P: q/k/v projections + RoPE ----------------
        with ExitStack() as ctx:
            consts = ctx.enter_context(tc.tile_pool(name="pconsts", bufs=1))
            xt_pool = ctx.enter_context(tc.tile_pool(name="xt", bufs=2))
            cs_pool = ctx.enter_context(tc.tile_pool(name="cs", bufs=2))
            ps_pool = ctx.enter_context(
                tc.tile_pool(name="pps", bufs=5, space="PSUM"))
            sw_pool = ctx.enter_context(
                tc.tile_pool(name="swps", bufs=3, space="PSUM"))
            tmp_pool = ctx.enter_context(tc.tile_pool(name="ptmp", bufs=2))

            p2t_sb = consts.tile([128, 128], F32R, tag="p2t")
            nc.sync.dma_start(out=p2t_sb, in_=ap["p2t"])
            wq_sb = consts.tile([128, CCH, EC], F32R, tag="wq")
            wk_sb = consts.tile([128, CCH, EC], F32R, tag="wk")
            wk_src = ap["wkT"].rearrange("(c p) e -> p c e", p=128)

            # first t-block's operands first: interleave wk chunk c with
            # xt chunk c so the k-projection accumulation starts at ~2us
            xt0 = xt_pool.tile([128, CCH, TB], F32R, tag="xt")
            for c in range(CCH):
                nc.sync.dma_start(out=wk_sb[:, c, :], in_=wk_src[:, c, :])
                nc.sync.dma_start(out=xt0[:, c, :], in_=xTr[:, c, 0:TB])
            cos0 = cs_pool.tile([128, TB], F32, tag="cos")
            sin0 = cs_pool.tile([128, TB], F32, tag="sin")
            nc.sync.dma_start(out=cos0, in_=ap["cosb"][:, 0:TB])
            nc.sync.dma_start(out=sin0, in_=ap["sinb"][:, 0:TB])
            for name, t_ in (("wqT", wq_sb), ("wvT", wv_sb)):
                src = ap[name].rearrange("(c p) e -> p c e", p=128)
                for c in range(CCH):
                    nc.sync.dma_start(out=t_[:, c, :], in_=src[:, c, :])


            for tb in range(NTB):
                ts = slice(tb * TB, (tb + 1) * TB)
                if tb == 0:
                    xt, cos_sb, sin_sb = xt0, cos0, sin0
                else:
                    xt = xt_pool.tile([128, CCH, TB], F32R, tag="xt")
                    for c in range(CCH):
                        nc.sync.dma_start(out=xt[:, c, :], in_=xTr[:, c, ts])
                    cos_sb = cs_pool.tile([128, TB], F32, tag="cos")
                    sin_sb = cs_pool.tile([128, TB], F32, tag="sin")
                    nc.sync.dma_start(out=cos_sb, in_=ap["cosb"][:, ts])
                    nc.sync.dma_start(out=sin_sb, in_=ap["sinb"][:, ts])

                # k/q: out[e_chunk 128, t 512], weights stationary; K first
                # so attention (which needs both) can start earliest.
                # RoPE for block m is deferred behind block m+1's projection
                # matmuls: the swap matmul waits on the DVE evac, and PE is
                # in-order, so emitting it immediately would stall PE.
                def emit_rope(dst, m):
                    sw = sw_pool.tile([128, TB], F32, tag="sw")
                    nc.tensor.matmul(sw[:], p2t_sb[:], dst[:, m, ts],
                                     start=True, stop=True)
                    t1 = tmp_pool.tile([128, TB], F32, tag="t1")
                    nc.vector.tensor_mul(t1[:], dst[:, m, ts], cos_sb[:])
                    t2 = tmp_pool.tile([128, TB], F32, tag="t2")
                    nc.vector.tensor_mul(t2[:], sw[:], sin_sb[:])
                    nc.vector.tensor_add(dst[:, m, ts], t1[:], t2[:])

                pending_rope = []
                for w_sb, dst in ((wk_sb, kT), (wq_sb, qT)):
                    for m in range(OCH):
                        pp = ps_pool.tile([128, TB], F32, tag="pp")
                        for c in range(CCH):
                            nc.tensor.matmul(
                                pp[:], w_sb[:, c, m * 128:(m + 1) * 128],
                                xt[:, c, :], start=(c == 0),
                                stop=(c == CCH - 1))
                        nc.vector.tensor_copy(dst[:, m, ts], pp[:])
                        pending_rope.append((dst, m))
                        if len(pending_rope) > 1:
                            emit_rope(*pending_rope.pop(0))
                for args in pending_rope:
                    emit_rope(*args)

                for st in range(TB // 128):
                    emit_vproj(tb * (TB // 128) + st, xt, st * 128,
                               ps_pool, "pp")

        # ---------------- Phases A+O: attention + out-proj ----------------
        with ExitStack() as ctx:
            consts = ctx.enter_context(tc.tile_pool(name="aconsts", bufs=1))
            oT = ctx.enter_context(
                tc.tile_pool(name="oT", bufs=1)).tile(
                    [128, OCH, T], F32R, tag="oT")
            s_pool = ctx.enter_context(
                tc.tile_pool(name="sps", bufs=5, space="PSUM"))
            o_pool = ctx.enter_context(
                tc.tile_pool(name="ops", bufs=2, space="PSUM"))
            y_pool = ctx.enter_context(
                tc.tile_pool(name="yps", bufs=1, space="PSUM"))
            e_pool = ctx.enter_context(tc.tile_pool(name="exp", bufs=6))
            r_pool = ctx.enter_context(tc.tile_pool(name="rcp", bufs=3))
            b_pool = ctx.enter_context(tc.tile_pool(name="bcast", bufs=3))
            ysb_pool = ctx.enter_context(tc.tile_pool(name="ysb", bufs=3))

            wo_sb = consts.tile([128, OCH, E], F32R, tag="wo")
            nc.sync.dma_start(
                out=wo_sb, in_=ap["woT"].rearrange("(c p) e -> p c e", p=128))

            def emit_yproj(qb, st, eh):
                tt = qb * (TB // 128) + st
                tsl = slice(tt * 128, (tt + 1) * 128)
                y_ps = y_pool.tile([128, 512], F32, tag="y")
                for c in range(OCH):
                    nc.tensor.matmul(
                        y_ps[:], oT[:, c, tsl],
                        wo_sb[:, c, eh * 512:(eh + 1) * 512],
                        start=(c == 0), stop=(c == OCH - 1))
                y_sb = ysb_pool.tile([128, 512], F32, tag="ysb")
                nc.vector.tensor_copy(y_sb[:], y_ps[:])
                nc.sync.dma_start(
                    out=y[tsl, eh * 512:(eh + 1) * 512], in_=y_sb[:])

            pending_y = []
            for qb in range(NTB):
                qs = slice(qb * TB, (qb + 1) * TB)
                nkv = (qb + 1) * (TB // KC)
                for h in range(HPC):
                    po = (h % 2) * 64
                    ch = h // 2
                    o_ps = o_pool.tile([D + 1, TB], F32, tag="o")

                    def emit_qk(kc):
                        # columns < 128j of a diagonal chunk are fully
                        # masked: skip them when the rest stays >= 256
                        # wide (f32r full rate); j=3 stays full width.
                        j = kc - 4 * qb
                        co = 128 * j if j in (1, 2) else 0
                        s_ps = s_pool.tile([128, TB], F32, tag="s")
                        nc.tensor.matmul(
                            s_ps[:, co:],
                            kT[po:po + D, ch, kc * KC:(kc + 1) * KC],
                            qT[po:po + D, ch, qb * TB + co:(qb + 1) * TB],
                            start=True, stop=True)
                        return s_ps

                    def emit_exp(kc, s_ps):
                        e_sb = e_pool.tile([128, TB], F32R, tag="e")
                        j = kc - 4 * qb
                        if j <= 0:
                            nc.scalar.activation(e_sb[:], s_ps[:],
                                                 AF.Exp, scale=0.125)
                        else:
                            # columns < 128j are fully masked: skip the
                            # exp there; affine_select zero-fills them
                            nc.scalar.activation(
                                e_sb[:, 128 * j:], s_ps[:, 128 * j:],
                                AF.Exp, scale=0.125)
                        if j >= 0:
                            # zero the not-yet-visible triangle:
                            # keep where q - k - 128j >= 0
                            nc.gpsimd.affine_select(
                                out=e_sb[:], in_=e_sb[:],
                                compare_op=mybir.AluOpType.is_ge,
                                fill=0.0, base=-128 * j,
                                pattern=[[1, TB]], channel_multiplier=-1)
                        return e_sb

                    def emit_av(kc, e_sb):
                        j = kc - 4 * qb
                        co = 128 * j if j in (1, 2) else 0
                        nc.tensor.matmul(o_ps[:, co:], vv[:, kc, h, :],
                                         e_sb[:, co:],
                                         start=(kc == 0),
                                         stop=(kc == nkv - 1))

                    # software pipeline with one-group lookahead (G=2):
                    # PE queue order QK(g) .. QK(g+1), AV(g) keeps PE from
                    # stalling on group g's exps; s_pool=5 covers 2 groups
                    G = 2
                    n_g = nkv // G
                    s_cur = [emit_qk(kc) for kc in range(G)]
                    for g in range(n_g):
                        base = G * g
                        e_cur = [emit_exp(base + i, s_cur[i])
                                 for i in range(G)]
                        if g + 1 < n_g:
                            s_cur = [emit_qk(base + G + i) for i in range(G)]
                        for i in range(G):
                            emit_av(base + i, e_cur[i])
                    r_sb = r_pool.tile([1, TB], F32, tag="r")
                    nc.vector.reciprocal(r_sb[:], o_ps[D:D + 1, :])
                    rb_sb = b_pool.tile([D, TB], F32, tag="rb")
                    nc.gpsimd.partition_broadcast(rb_sb[:], r_sb[:])
                    nc.vector.tensor_mul(oT[po:po + D, ch, qs],
                                         o_ps[0:D, :], rb_sb[:])
                    # interleave one out-proj chunk of the previous q-block
                    # between heads so PE never runs a long out-proj burst
                    if pending_y:
                        emit_yproj(*pending_y.pop(0))

                while pending_y:
                    emit_yproj(*pending_y.pop(0))
                pending_y = [(qb, st, eh) for st in range(TB // 128)
                             for eh in range(2)]
            for args in pending_y:
                emit_yproj(*args)
    nc.compile()
    return nc


def _host_inputs(x, Wq, Wk, Wv, Wo):
    # rope tables in [e, t] layout, duplicated across the 2 heads of a chunk
    inv_freq = 1.0 / (ROPE_BASE ** (np.arange(0, D, 2, dtype=np.float64) / D))
    freqs = np.outer(np.arange(T, dtype=np.float64), inv_freq)  # [T, 32]
    emb = np.concatenate([freqs, freqs], axis=-1)               # [T, 64]
    cos1, sin1 = np.cos(emb).T, np.sin(emb).T                   # [64, T]
    cosb = np.concatenate([cos1, cos1], 0).astype(np.float32)   # [128, T]
    sinb = np.concatenate([sin1, sin1], 0).astype(np.float32)

    # rotate-half as lhsT: out = p2t.T @ q = R2 @ q
    R = np.zeros((64, 64), dtype=np.float32)
    for i in range(32):
        R[i, i + 32] = -1.0
        R[i + 32, i] = 1.0
    R2 = np.zeros((128, 128), dtype=np.float32)
    R2[0:64, 0:64] = R
    R2[64:128, 64:128] = R
    p2t = np.ascontiguousarray(R2.T)

    xTs = [_round_f32r(x[b].T) for b in range(B)]
    wmaps = []
    for hg in range(2):
        cols = slice(hg * EC, (hg + 1) * EC)
        wmaps.append({
            "wqT": _round_f32r(Wq[cols, :].T),
            "wkT": _round_f32r(Wk[cols, :].T),
            "wvT": _round_f32r(Wv[cols, :].T),
            "woT": _round_f32r(Wo[:, cols].T),
        })
    in_maps = []
    for c in range(N_CORES):
        b, hg = c // 2, c % 2
        in_maps.append({
            "xT": xTs[b], "p2t": p2t, "cosb": cosb, "sinb": sinb,
            **wmaps[hg],
        })
    return in_maps


def kernel(x, causal_mask, Wq, Wk, Wv, Wo):
    global _NC
    x = np.asarray(x, dtype=np.float32)
    Wq = np.asarray(Wq, dtype=np.float32)
    Wk = np.asarray(Wk, dtype=np.float32)
    Wv = np.asarray(Wv, dtype=np.float32)
    Wo = np.asarray(Wo, dtype=np.float32)
    if _NC is None:
        _NC = _build()
    in_maps = _host_inputs(x, Wq, Wk, Wv, Wo)
    try:
        res = run_bass_kernel_spmd(_NC, in_maps, list(range(N_CORES)))
    except Exception:
        # transient NRT/device hiccups recover on retry
        import time
        time.sleep(2)
        res = run_bass_kernel_spmd(_NC, in_maps, list(range(N_CORES)))
    out = np.empty((B, T, E), dtype=np.float32)
    for b in range(B):
        out[b] = res.results[2 * b]["y"] + res.results[2 * b + 1]["y"]
    return out
